# revision 18
# baseline (speedup 1.0000x reference)
"""Trainium2 Bass kernel for nn_MultiHeadAttention_91190745628911 (v2).

Full (unsharded) inputs in, full output out. Sharding: data parallel on
batch (2) x tensor parallel on heads (4 groups of 4 heads) = 8 cores.
Each core computes LN + its QKV slice + attention for its 4 heads + a
partial output projection; the host sums the 4 partials per batch and
transposes back to (seq, batch, hidden).

v2 redesign vs baseline:
- all matmuls in bf16/f16 (fp32r streams ~2.4x slower and throttles)
- host pre-transposes x -> xT and folds gamma into the weights, so the
  256 PE transposes of the LN output are gone; LN is applied as
  x*rstd pre-scale (DVE) plus a rank-1 correction folded into the
  PSUM evacuation (using host-precomputed weight column sums)
- no activation-table ping-pong: Sqrt+DVE-reciprocal for rstd,
  DVE reciprocal_approx_fast + GpSimd partition_broadcast for the
  softmax normalization (no Ln/Exp pairs, no DRAM bounces)
- mask multiplies alternate between DVE and the (idle) GpSimd engine
- QK matmuls contract over 64 partitions (no zero-padded q stripes)
- phase A (stats) and phase B (QKV) pipelined per 512-row s-group

Self-contained: hardcodes all shapes from the problem spec.
"""
import numpy as np
import ml_dtypes
from contextlib import ExitStack

import concourse.bass as bass
import concourse.tile as tile
from concourse import bacc, mybir
from concourse.bass_utils import run_bass_kernel_spmd
from concourse.tile_rust import add_dep_helper

F32 = mybir.dt.float32
BF16 = mybir.dt.float16  # fp16: same PE rate as bf16, 3 extra mantissa bits
F16 = mybir.dt.float16

SEQ, BATCH, HIDDEN = 2048, 2, 1024
NUM_HEADS, HEAD_DIM = 16, 64
N_CORES = 8
CORES_PER_BATCH = 4
HEADS_PER_CORE = NUM_HEADS // CORES_PER_BATCH  # 4
LN_EPS = 1e-6


class Cfg:
    def __init__(self, S=SEQ, E=HIDDEN, NH=HEADS_PER_CORE, HD=HEAD_DIM):
        self.S, self.E, self.NH, self.HD = S, E, NH, HD
        self.EC = E // 128              # e-chunks (8)
        self.ST = S // 128              # s-tiles (16)
        self.F = NH * HD                # features per core per projection
        self.FC = self.F // 128         # head-pair tiles (2)
        self.KC = S // 128              # k-chunks (16)
        self.QHALF = 1024
        self.NQH = S // self.QHALF      # 2
        self.QB = 512
        self.NQB = self.QHALF // self.QB
        self.SB = 512                   # s-block for projections
        self.NSB = S // self.SB         # 4
        self.TPG = self.SB // 128       # s-tiles per group (4)


def build_nc(cfg: Cfg, dbg: bool = False):
    nc = bacc.Bacc("TRN2", target_bir_lowering=False, debug=False)
    S, E, NH, HD = cfg.S, cfg.E, cfg.NH, cfg.HD
    EC, ST, F, FC, KC = cfg.EC, cfg.ST, cfg.F, cfg.FC, cfg.KC
    QHALF, NQH, QB, NQB = cfg.QHALF, cfg.NQH, cfg.QB, cfg.NQB
    SB, NSB, TPG = cfg.SB, cfg.NSB, cfg.TPG

    x_d = nc.dram_tensor("x", [S, E], BF16, kind="ExternalInput")
    xT_d = nc.dram_tensor("xT", [E, S], BF16, kind="ExternalInput")
    wq_d = nc.dram_tensor("wq", [E, F], BF16, kind="ExternalInput")
    wk_d = nc.dram_tensor("wk", [E, F], BF16, kind="ExternalInput")
    wv_d = nc.dram_tensor("wv", [E, F], BF16, kind="ExternalInput")
    wo_d = nc.dram_tensor("wo", [F, E], BF16, kind="ExternalInput")
    csum_d = nc.dram_tensor("csum", [2, F], F32, kind="ExternalInput")
    cvrow_d = nc.dram_tensor("cvrow", [1, F], F32, kind="ExternalInput")
    bias_d = nc.dram_tensor("biasr", [1, 3 * F], BF16, kind="ExternalInput")
    ident_d = nc.dram_tensor("ident", [128, 128], F32, kind="ExternalInput")
    maskT_d = nc.dram_tensor("maskT", [S, S], F16, kind="ExternalInput")
    out_d = nc.dram_tensor("outT", [E, S], BF16, kind="ExternalOutput")
    scr_d = nc.dram_tensor("scr", [NSB, 2 * TPG, 128], BF16)  # stats bounce

    with tile.TileContext(nc) as tc, ExitStack() as ctx:
        singles = ctx.enter_context(tc.tile_pool(name="singles", bufs=1))
        big = ctx.enter_context(tc.tile_pool(name="big", bufs=1))

        # ---------- persistent activation storages ----------
        qT = big.tile([128, FC, S], BF16)       # [2 heads x 64, hp, s]
        kT = big.tile([128, FC, S], BF16)
        v_sb = big.tile([128, KC, NH, 66], F16)
        ctxT = big.tile([128, FC, S], BF16)
        mh0 = big.tile([128, KC, QHALF], F16, tag="mask0")
        mh1 = big.tile([128, KC, QHALF], F16, tag="mask1")
        mhs = [mh0, mh1]

        # ---------- upfront DMAs ----------
        for qh in range(NQH):
            nc.sync.dma_start(
                out=mhs[qh],
                in_=maskT_d.ap()[:, qh * QHALF:(qh + 1) * QHALF].rearrange(
                    "(kc p) q -> p kc q", p=128))
        w_sbs = {}
        for name, d in (("q", wq_d), ("k", wk_d), ("v", wv_d)):
            w_sb = singles.tile([128, EC, F], BF16, tag=f"w{name}")
            nc.sync.dma_start(
                out=w_sb, in_=d.ap().rearrange("(ec p) f -> p ec f", p=128))
            w_sbs[name] = w_sb
        wo_sb = singles.tile([128, FC, E], BF16)
        nc.sync.dma_start(
            out=wo_sb, in_=wo_d.ap().rearrange("(fc p) e -> p fc e", p=128))
        csum_sb = singles.tile([128, 2, FC], F32)
        nc.sync.dma_start(
            out=csum_sb, in_=csum_d.ap().rearrange("k (fc p) -> p k fc", p=128))
        cvrow_sb = singles.tile([1, F], F32)
        nc.sync.dma_start(out=cvrow_sb, in_=cvrow_d.ap())
        bias_sb = singles.tile([1, 3, F], BF16)
        nc.sync.dma_start(out=bias_sb,
                          in_=bias_d.ap().rearrange("o (k f) -> o k f", k=3))
        ident_sb = singles.tile([128, 128], F32)
        nc.sync.dma_start(out=ident_sb, in_=ident_d.ap())
        eps_sb = singles.tile([128, 1], F32)
        nc.vector.memset(eps_sb, LN_EPS)
        ones_row = singles.tile([1, QB], BF16)
        nc.vector.memset(ones_row, 1.0)
        cv_bc = singles.tile([128, F], F32)
        nc.gpsimd.partition_broadcast(cv_bc, cvrow_sb)
        nc.gpsimd.memset(v_sb[:, :, :, 64:66], 1.0)

        # per-token LN stats per s-group g: cols [g, 0:TPG] = rstd,
        # cols [g, TPG:2*TPG] = -mean*rstd (tile j within group)
        mr_nat = singles.tile([128, NSB, 2 * TPG], F32)
        rT_bc = singles.tile([128, S], BF16)
        m2T_bc = singles.tile([128, S], BF16)

        with ExitStack() as ab_ctx:
            phA = ab_ctx.enter_context(tc.tile_pool(name="phA", bufs=2))
            phAst = ab_ctx.enter_context(tc.tile_pool(name="phAst", bufs=4))
            psA = ab_ctx.enter_context(
                tc.tile_pool(name="psA", bufs=2, space="PSUM"))
            psB = ab_ctx.enter_context(
                tc.tile_pool(name="psB", bufs=3, space="PSUM"))
            xT_sb = ab_ctx.enter_context(
                tc.tile_pool(name="xTp", bufs=1)).tile([128, EC, S], BF16)

            for ec in range(EC):
                nc.sync.dma_start(
                    out=xT_sb[:, ec, :],
                    in_=xT_d.ap()[ec * 128:(ec + 1) * 128, :])

            n_sub = E // 512

            def phase_a(g):
                # stats for the TPG s-tiles of group g, then transpose the
                # per-tile (rstd, -mean*rstd) columns and broadcast them
                # along partitions for this 512-token block; finally
                # pre-scale xT by rstd in place.
                x_g = phA.tile([128, TPG, E], BF16, tag="x")
                nc.sync.dma_start(
                    out=x_g,
                    in_=x_d.ap()[g * SB:(g + 1) * SB, :].rearrange(
                        "(t p) e -> p t e", p=128))
                for j in range(TPG):
                    t = g * TPG + j
                    st = phAst.tile([128, n_sub, nc.vector.BN_STATS_DIM], F32,
                                    tag="st")
                    xr = x_g[:, j, :].rearrange("p (a b) -> p a b", a=n_sub)
                    for i in range(n_sub):
                        nc.vector.bn_stats(out=st[:, i, :], in_=xr[:, i, :])
                    mv = phAst.tile([128, nc.vector.BN_AGGR_DIM], F32, tag="mv")
                    nc.vector.bn_aggr(out=mv, in_=st)
                    sq = phAst.tile([128, 1], F32, tag="sq")
                    nc.scalar.activation(sq, mv[:, 1:2],
                                         mybir.ActivationFunctionType.Sqrt,
                                         bias=eps_sb[:], scale=1.0)
                    nc.vector.reciprocal_approx_fast(
                        out=mr_nat[:, g, j:j + 1], in_=sq)
                    nc.vector.scalar_tensor_tensor(
                        out=mr_nat[:, g, TPG + j:TPG + j + 1],
                        in0=mv[:, 0:1], scalar=-1.0,
                        in1=mr_nat[:, g, j:j + 1],
                        op0=mybir.AluOpType.mult, op1=mybir.AluOpType.mult)
                tr = psA.tile([2 * TPG, 128], F32, tag="tr")
                nc.tensor.transpose(tr, mr_nat[:, g, :], ident_sb)
                trT = phAst.tile([2 * TPG, 128], BF16, tag="trT")
                nc.vector.tensor_copy(trT, tr)
                wr = nc.sync.dma_start(out=scr_d.ap()[g], in_=trT)
                sl = slice(g * SB, (g + 1) * SB)
                # broadcast (stride-0 partition read) back from DRAM
                base = g * 2 * TPG * 128
                rd0 = nc.sync.dma_start(
                    out=rT_bc[:, sl].rearrange("p (t q) -> p t q", q=128),
                    in_=bass.AP(tensor=scr_d, offset=base,
                                ap=[[0, 128], [128, TPG], [1, 128]]))
                rd1 = nc.sync.dma_start(
                    out=m2T_bc[:, sl].rearrange("p (t q) -> p t q", q=128),
                    in_=bass.AP(tensor=scr_d, offset=base + TPG * 128,
                                ap=[[0, 128], [128, TPG], [1, 128]]))
                add_dep_helper(rd0.ins, wr.ins, reason="stats RAW")
                add_dep_helper(rd1.ins, wr.ins, reason="stats RAW")
                for ec in range(EC):
                    nc.vector.tensor_tensor(
                        out=xT_sb[:, ec, sl], in0=xT_sb[:, ec, sl],
                        in1=rT_bc[:, sl], op=mybir.AluOpType.mult)

            def phase_b(g):
                sl = slice(g * SB, (g + 1) * SB)
                for ki, name in ((0, "q"), (1, "k")):
                    w_sb = w_sbs[name]
                    tgt = qT if name == "q" else kT
                    for fc in range(FC):
                        ps = psB.tile([128, SB], F32, tag="qk_ps")
                        for ec in range(EC):
                            nc.tensor.matmul(
                                ps, lhsT=w_sb[:, ec, fc * 128:(fc + 1) * 128],
                                rhs=xT_sb[:, ec, sl],
                                start=(ec == 0), stop=False)
                        nc.tensor.matmul(
                            ps, lhsT=bias_sb[0:1, ki, fc * 128:(fc + 1) * 128],
                            rhs=ones_row[0:1, :], start=False, stop=True)
                        nc.vector.scalar_tensor_tensor(
                            out=tgt[:, fc, sl], in0=m2T_bc[:, sl],
                            scalar=csum_sb[:, ki, fc:fc + 1], in1=ps,
                            op0=mybir.AluOpType.mult, op1=mybir.AluOpType.add)
                w_sb = w_sbs["v"]
                for j in range(TPG):
                    t = g * TPG + j
                    ps = psB.tile([128, F], F32, tag="v_ps", bufs=2)
                    for ec in range(EC):
                        nc.tensor.matmul(
                            ps, lhsT=xT_sb[:, ec, t * 128:(t + 1) * 128],
                            rhs=w_sb[:, ec, :], start=(ec == 0), stop=False)
                    nc.tensor.matmul(
                        ps, lhsT=ones_row[0:1, 0:128], rhs=bias_sb[0:1, 2, :],
                        start=False, stop=True)
                    nc.vector.scalar_tensor_tensor(
                        out=v_sb[:, t, :, 0:64], in0=cv_bc,
                        scalar=mr_nat[:, g, TPG + j:TPG + j + 1], in1=ps,
                        op0=mybir.AluOpType.mult, op1=mybir.AluOpType.add)

            phase_a(0)
            for g in range(NSB):
                if g + 1 < NSB:
                    phase_a(g + 1)
                phase_b(g)

        # ---------- Phase C: attention ----------
        with ExitStack() as c_ctx:
            phC = c_ctx.enter_context(tc.tile_pool(name="phC", bufs=2))
            psRing = c_ctx.enter_context(
                tc.tile_pool(name="psRing", bufs=1, space="PSUM"))
            psCtx = c_ctx.enter_context(
                tc.tile_pool(name="psCtx", bufs=1, space="PSUM"))
            for qh in range(NQH):
                mh = mhs[qh]
                qsl = slice(qh * QHALF, (qh + 1) * QHALF)
                for h in range(NH):
                    hp, hh = h // 2, h % 2
                    pr = slice(hh * 64, hh * 64 + 64)
                    ctx_t = psCtx.tile([128, QHALF], F32, tag="ctx")
                    ring = psRing.tile([128, 3, QHALF], F32, tag="ring")

                    def flush(kc_lo, nk, ring=ring, ctx_t=ctx_t, mh=mh, h=h):
                        s0 = kc_lo % 3
                        at = phC.tile([128, 2, QHALF], F16, tag="attn", bufs=4)
                        nc.scalar.activation(
                            at[:, 0:nk, :], ring[:, s0:s0 + nk, :],
                            mybir.ActivationFunctionType.Exp)
                        for j in range(nk):
                            kcj = kc_lo + j
                            eng = nc.vector if (kcj % 2 == 0) else nc.gpsimd
                            eng.tensor_tensor(
                                out=at[:, j, :], in0=at[:, j, :],
                                in1=mh[:, kcj, :], op=mybir.AluOpType.mult)
                            for qb in range(NQB):
                                nc.tensor.matmul(
                                    ctx_t[0:66, qb * QB:(qb + 1) * QB],
                                    lhsT=v_sb[:, kcj, h, :],
                                    rhs=at[:, j, qb * QB:(qb + 1) * QB],
                                    start=(kcj == 0), stop=(kcj == KC - 1))

                    for kc in range(KC):
                        slot = kc % 3
                        for qb in range(NQB):
                            nc.tensor.matmul(
                                ring[:, slot, qb * QB:(qb + 1) * QB],
                                lhsT=kT[pr, hp, kc * 128:(kc + 1) * 128],
                                rhs=qT[pr, hp,
                                       qh * QHALF + qb * QB:
                                       qh * QHALF + (qb + 1) * QB],
                                start=True, stop=True)
                        if slot == 1:
                            flush(kc - 1, 2)
                        elif slot == 2:
                            flush(kc, 1)
                    if (KC - 1) % 3 == 0:
                        flush(KC - 1, 1)

                    # normalize: ctx / sums, sums in psum row 64
                    stgU = phC.tile([66, QHALF], F32, tag="stgU", bufs=3)
                    nc.vector.tensor_copy(stgU, ctx_t[0:66, :])
                    sums0 = phC.tile([1, QHALF], F32, tag="sums0", bufs=2)
                    nc.sync.dma_start(out=sums0, in_=stgU[64:65, :])
                    recf = phC.tile([1, QHALF], F32, tag="recf", bufs=2)
                    nc.vector.reciprocal_approx_fast(out=recf, in_=sums0)
                    rbc = phC.tile([64, QHALF], F32, tag="rbc", bufs=2)
                    nc.gpsimd.partition_broadcast(rbc, recf)
                    if dbg and qh == 0 and h == 0:
                        for nm, t in (("d_stgU", stgU), ("d_recf", recf),
                                      ("d_rbc", rbc)):
                            dd = nc.dram_tensor(
                                nm, list(t.shape), F32, kind="ExternalOutput")
                            nc.sync.dma_start(out=dd.ap(), in_=t)
                    if hh == 0:
                        nc.vector.scalar_tensor_tensor(
                            out=ctxT[0:64, hp, qsl], in0=stgU[0:64, :],
                            scalar=1.0, in1=rbc,
                            op0=mybir.AluOpType.mult, op1=mybir.AluOpType.mult)
                    else:
                        stg = phC.tile([64, QHALF], BF16, tag="stg", bufs=2)
                        nc.vector.scalar_tensor_tensor(
                            out=stg, in0=stgU[0:64, :], scalar=1.0, in1=rbc,
                            op0=mybir.AluOpType.mult, op1=mybir.AluOpType.mult)
                        nc.sync.dma_start(out=ctxT[64:128, hp, qsl], in_=stg)

        if dbg:
            for nm, t in (("d_qT", qT), ("d_kT", kT), ("d_ctxT", ctxT)):
                dd = nc.dram_tensor(nm, [128, FC * S], BF16,
                                    kind="ExternalOutput")
                nc.sync.dma_start(out=dd.ap().rearrange(
                    "p (a b) -> p a b", a=FC), in_=t)
            dv = nc.dram_tensor("d_v", [128, KC * NH * 66], F16,
                                kind="ExternalOutput")
            nc.sync.dma_start(out=dv.ap().rearrange(
                "p (a b c) -> p a b c", a=KC, b=NH), in_=v_sb)
            for nm, t in (("d_rT", rT_bc), ("d_m2T", m2T_bc)):
                dd = nc.dram_tensor(nm, [128, S], BF16, kind="ExternalOutput")
                nc.sync.dma_start(out=dd.ap(), in_=t)

        # ---------- Phase D: output projection (transposed) ----------
        with tc.tile_pool(name="phD", bufs=4) as phD, \
             tc.tile_pool(name="psD", bufs=4, space="PSUM") as psD:
            for ec in range(EC):
                for sb in range(NSB):
                    ps = psD.tile([128, SB], F32, tag="o_ps")
                    for fc in range(FC):
                        nc.tensor.matmul(
                            ps, lhsT=wo_sb[:, fc, ec * 128:(ec + 1) * 128],
                            rhs=ctxT[:, fc, sb * SB:(sb + 1) * SB],
                            start=(fc == 0), stop=(fc == FC - 1))
                    o_t = phD.tile([128, SB], BF16, tag="o_sb")
                    nc.vector.tensor_copy(o_t, ps)
                    nc.sync.dma_start(
                        out=out_d.ap()[ec * 128:(ec + 1) * 128,
                                       sb * SB:(sb + 1) * SB],
                        in_=o_t)

    nc.compile()
    return nc


_CACHED = {}


def _get_nc():
    if "nc" not in _CACHED:
        _CACHED["nc"] = build_nc(Cfg())
    return _CACHED["nc"]


def make_in_maps(cfg, inputs_q, mask, ln_scale, ln_bias, w_qkv, w_out,
                 n_cores=N_CORES, cores_per_batch=CORES_PER_BATCH):
    bf16 = np.float16
    ident = np.eye(128, dtype=np.float32)
    gamma = np.asarray(ln_scale, dtype=np.float32)
    beta = np.asarray(ln_bias, dtype=np.float32)
    in_maps = []
    for c in range(n_cores):
        b = c // cores_per_batch
        g = c % cores_per_batch
        f0 = g * cfg.F
        f1 = f0 + cfg.F
        x_c = np.ascontiguousarray(inputs_q[:, b, :], dtype=np.float32)
        # fold gamma into the projection weights (ln*g+b @ w)
        wq = np.asarray(w_qkv[:, 0, f0:f1], dtype=np.float32) * gamma[:, None]
        wk = np.asarray(w_qkv[:, 1, f0:f1], dtype=np.float32) * gamma[:, None]
        wv = np.asarray(w_qkv[:, 2, f0:f1], dtype=np.float32) * gamma[:, None]
        biasr = np.stack([beta @ wq, beta @ wk, beta @ wv])  # [3, F]
        csum = np.stack([wq.sum(0), wk.sum(0)])              # [2, F]
        cvrow = wv.sum(0)[None, :]                           # [1, F]
        maskT_c = np.ascontiguousarray((~mask[b, 0]).T).astype(np.float16)
        in_maps.append({
            "x": x_c.astype(bf16),
            "xT": np.ascontiguousarray(x_c.T).astype(bf16),
            "wq": wq.astype(bf16),
            "wk": wk.astype(bf16),
            "wv": wv.astype(bf16),
            "wo": np.ascontiguousarray(w_out[f0:f1, :]).astype(bf16),
            "csum": np.ascontiguousarray(csum),
            "cvrow": np.ascontiguousarray(cvrow),
            "biasr": biasr.astype(bf16),
            "ident": ident,
            "maskT": maskT_c,
        })
    return in_maps


def combine_outputs(results):
    outTs = np.stack([np.asarray(results[c]["outT"], dtype=np.float32)
                      for c in range(N_CORES)])
    out = outTs.reshape(BATCH, CORES_PER_BATCH, HIDDEN, SEQ).sum(axis=1)
    return np.ascontiguousarray(out.transpose(2, 0, 1)).astype(np.float32)


def kernel(inputs_q, mask, ln_scale, ln_bias, w_qkv, w_out):
    nc = _get_nc()
    in_maps = make_in_maps(Cfg(), inputs_q, mask, ln_scale, ln_bias,
                           w_qkv, w_out)
    res = run_bass_kernel_spmd(nc, in_maps, list(range(N_CORES)))
    return combine_outputs(res.results)
